# revision 49
# baseline (speedup 1.0000x reference)
"""CapsuleLayer dynamic-routing kernel for 8 Trainium2 NeuronCores.

Problem: u_hat[b,i,j,e] = einsum('bid,ijde->bije', x, W) with
B=64, I=2304, D=8, J=32, E=16, followed by NUM_ROUTING=3 softmax
routing iterations.  Output V = squash(S_2) with shape [B, J, E].

Strategy (data-parallel over batch, 8 b per core):
 - Host pre-lays W into fp16 tiles [G=144, 128, 512] with partition
   p = (i_loc*8 + d) and free f = (e*32 + j); W streams through SBUF once.
 - A block-diagonal lhsT (built on-chip from x/J with a mask multiply)
   makes ONE matmul produce u_hat for 16 i's x 8 b x (all j,e) per group;
   a second matmul per group accumulates S_0 = (1/J) sum_i u_hat in PSUM.
 - u_hat stays resident in SBUF as fp16 [128=(il,b), G*512=(g,(e,j))] —
   it never round-trips to HBM.
 - Each routing iteration (DVE TensorTensor ops run in 2x mode for packed
   fp16): P = u_hat*V_rep, in-place e-reduction tree for the agreement
   logits, one batched ACT exp for softmax, T = u_hat*c into the dead P
   tile, and PE contracts over i with a constant indicator lhsT,
   accumulating S in PSUM.  Two-stage software pipelining plus triple
   buffering of the product tile keeps DVE (the bottleneck) saturated.
"""

import sys

import numpy as np

sys.path.insert(0, "/opt/trn_rl_repo")

B, I, D, J, E = 64, 2304, 8, 32, 16
NC_CORES = 8
BS = B // NC_CORES          # 8 batch elements per core
IL = 16                     # i's per group
G = I // IL                 # 144 groups
F = J * E                   # 512 free elements per group
GB = 6                      # groups per batched DVE macro-op
P_BUFS = 5                  # product-tile buffering
W_BUFS = 4                  # W-stream buffering
SM_BUFS = 2                 # softmax small-tile buffering
GP_EVERY = 0                # offload P-mult of every Nth macro to GPSIMD (0=off)
# Pool offload: per-macro number of trailing groups whose P/tree/T ops run
# on GPSIMD instead of DVE (list cycled over macros; 0 = all-DVE).
GP_GSPLIT = (1, 1, 1, 2)
# Whole-macro Pool assignment: cycled truth-list; macros marked 1 run their
# entire TT chain (P, tree, logit-add, cc, T) on GPSIMD so the slower Pool
# engine is never on the DVE pipeline's critical path.  (Measured worse than
# the fine-grained GP_GSPLIT on this problem; left available but off.)
GP_MACRO = ()
GP_STAGEA_ONLY = True       # Pool macros offload only the P/tree/logit chain;
                            # cc/T (feeding PE S-matmuls) stay on DVE
SPLIT_SMALLS = False        # split a2/lg/rc/cc per owning engine
CC_VIA_ACT = False          # c = exp(lg - ln(sumexp)) on ACT instead of
                            # reciprocal+multiply on (bottleneck) DVE
                            # (measured worse: ACT becomes the bottleneck)
PIPE_DEPTH = 1              # stage_b lag (slots) behind stage_a
EXP_SLOT = 0                # exp emission lag (slots) behind stage_a
# phase-1 psum-drain engine rotation (GPSIMD cannot access PSUM, so only
# ACT/DVE are legal here)
DRAIN_ROT = ("act", "dve", "act")
NUM_ROUTING = 3

_CACHE = {}


def _force_single_act_table(bacc, mybir):
    """Constrain the activation-table chooser to one set covering every
    function this kernel uses (Exp/Ln/Copy/Square), so no LoadActFuncSet
    reloads (~1.3us each on ACT) appear at routing-pass boundaries.  Set ids
    stay positional, so all other sets are emptied rather than removed."""
    if getattr(bacc, "_act_tables_forced", False):
        return
    orig = bacc.get_activation_tables
    AF = mybir.ActivationFunctionType
    need = {AF.Exp, AF.Ln, AF.Copy, AF.Square}

    def patched(arch):
        tabs = orig(arch)
        chosen = None
        for name, fns in tabs.items():
            if need <= set(fns):
                chosen = name
                break
        if chosen is not None:
            for name in list(tabs):
                if name != chosen:
                    tabs[name] = set()
        return tabs

    bacc.get_activation_tables = patched
    bacc._act_tables_forced = True


def _build_program(n_groups, nonzero_b0, n_passes=2, n_bodies=1):
    import concourse.bass as bass
    import concourse.mybir as mybir
    import concourse.tile as tile
    from concourse import bacc

    _force_single_act_table(bacc, mybir)

    fp16 = mybir.dt.float16
    bf16 = mybir.dt.bfloat16
    f32 = mybir.dt.float32

    nc = bacc.Bacc("TRN2", target_bir_lowering=False, debug=False)

    # register the squash-epsilon constant for activation bias
    eps_t = nc.alloc_sbuf_tensor("const-f32-eps", [128, 1], f32)
    nc.gpsimd.memset(eps_t.ap(), 1e-7)
    nc.const_aps.aps[(f32, 1e-7)] = eps_t.ap()
    nc.all_engine_barrier()

    g_ = n_groups
    wp = nc.dram_tensor("wp", [g_, 128, F], fp16, kind="ExternalInput").ap()
    xs0 = nc.dram_tensor("xs0", [128, g_, BS], fp16, kind="ExternalInput").ap()
    msk = nc.dram_tensor("msk", [128, 128], fp16, kind="ExternalInput").ap()
    ind = nc.dram_tensor("ind", [128, BS], fp16, kind="ExternalInput").ap()
    vind = nc.dram_tensor("vind", [BS, 128], fp16, kind="ExternalInput").ap()
    if nonzero_b0:
        wp0 = nc.dram_tensor("wp0", [g_, 128, F], fp16, kind="ExternalInput").ap()
        b0p = nc.dram_tensor("b0p", [128, g_ * J], f32, kind="ExternalInput").ap()
    v_out = nc.dram_tensor("v_out", [BS, F], f32, kind="ExternalOutput").ap()

    from contextlib import ExitStack

    with tile.TileContext(nc) as tc:
        for _body in range(n_bodies):
            _sfx = "" if _body == 0 else "@%d" % _body
            with ExitStack() as ctx:
                ent = ctx.enter_context
                uhat_pool = ent(tc.tile_pool(name="uhat" + _sfx, bufs=1))
                alog_pool = ent(tc.tile_pool(name="alog" + _sfx, bufs=1))
                cst_pool = ent(tc.tile_pool(name="cst" + _sfx, bufs=1))
                sm_pool = ent(tc.tile_pool(name="sm" + _sfx, bufs=SM_BUFS))
                vrep_pool = ent(tc.tile_pool(name="vrep" + _sfx, bufs=2))
                sq_pool = ent(tc.tile_pool(name="sq" + _sfx, bufs=1))
                s0_psum = ent(tc.tile_pool(name="s0ps" + _sfx, bufs=1, space="PSUM"))
                phase1 = ExitStack()
                xs0_pool = phase1.enter_context(tc.tile_pool(name="xs0p" + _sfx, bufs=1))
                w_pool = phase1.enter_context(tc.tile_pool(name="wstream" + _sfx, bufs=W_BUFS))
                l_pool = phase1.enter_context(tc.tile_pool(name="lstream" + _sfx, bufs=4))
                mm_psum = phase1.enter_context(
                    tc.tile_pool(name="mmps" + _sfx, bufs=3, space="PSUM")
                )
                # ---- persistent SBUF tensors ----
                uhat = uhat_pool.tile([128, g_ * F], fp16)       # (g,(e,j)) per part
                uv = uhat[:].rearrange("p (g f) -> p g f", g=g_)
                # iteration-1 logits in fp16: |logit| < ~30 so the absolute
                # quantization step is <= 0.016; the induced ~1.6% relative
                # c-weight noise averages out over I=2304 in S.  Keeping the
                # tile fp16 makes the a2/lg adds 2x-mode and frees 9KB of
                # SBUF per partition for deeper product-tile buffering.
                a1 = alog_pool.tile([128, g_ * J], fp16)
                a1v = a1[:].rearrange("p (g j) -> p g j", g=g_)
                xs0_sb = xs0_pool.tile([128, g_ * BS], fp16)
                xs0v = xs0_sb[:].rearrange("p (g b) -> p g b", g=g_)
                ind_sb = cst_pool.tile([128, BS], fp16)
                vind_sb = cst_pool.tile([BS, 128], fp16)
                msk_sb = cst_pool.tile([128, 128], fp16)
                if nonzero_b0:
                    b0_sb = alog_pool.tile([128, g_ * J], f32)
                    b0v = b0_sb[:].rearrange("p (g j) -> p g j", g=g_)

                nc.sync.dma_start(xs0_sb[:], xs0.rearrange("p g b -> p (g b)"))
                nc.sync.dma_start(ind_sb[:], ind)
                nc.sync.dma_start(vind_sb[:], vind)
                nc.sync.dma_start(msk_sb[:], msk)
                if nonzero_b0:
                    nc.sync.dma_start(b0_sb[:], b0p)

                # ---- phase 1: u_hat + S0 ----
                # W DMA in batches of GD groups.  The block-diagonal lhsT is
                # built on-chip from xs0 (= x/J) with a mask multiply, so u_hat
                # lands in PSUM scaled by 1/J and the psum->SBUF copy multiplies
                # by J.  Copies alternate between ACT and DVE.
                GD = 8
                mulJ = float(J)
                s0 = s0_psum.tile([BS, F], f32)
                assert g_ % GD == 0
                _mm = mybir.AluOpType.mult
                for gd in range(g_ // GD):
                    g0 = gd * GD
                    wt = w_pool.tile([128, GD * F], fp16)
                    wtv = wt[:].rearrange("p (g f) -> p g f", g=GD)
                    nc.sync.dma_start(wtv, wp[g0:g0 + GD].rearrange("g p f -> p g f"))
                    if nonzero_b0:
                        w0t = w_pool.tile([128, GD * F], fp16, tag="w0t")
                        w0tv = w0t[:].rearrange("p (g f) -> p g f", g=GD)
                        nc.sync.dma_start(
                            w0tv, wp0[g0:g0 + GD].rearrange("g p f -> p g f")
                        )
                    for h in range(GD // 2):
                        ps = mm_psum.tile([128, 2 * F], f32)
                        for k in range(2):
                            g = g0 + h * 2 + k
                            lt = l_pool.tile([128, 128], fp16)
                            xsb = xs0v[:, g][:, None, :].broadcast_to([128, IL, BS])
                            nc.vector.tensor_tensor(
                                lt[:].rearrange("p (i b) -> p i b", i=IL),
                                xsb, msk_sb[:].rearrange("p (i b) -> p i b", i=IL),
                                op=_mm,
                            )
                            nc.tensor.matmul(
                                ps[:, k * F:(k + 1) * F], lhsT=lt[:],
                                rhs=wtv[:, h * 2 + k], start=True, stop=True,
                            )
                            s0_rhs = w0tv[:, h * 2 + k] if nonzero_b0 else wtv[:, h * 2 + k]
                            nc.tensor.matmul(
                                s0[:], lhsT=xs0v[:, g], rhs=s0_rhs,
                                start=(g == 0), stop=(g == g_ - 1),
                            )
                        gg = g0 + h * 2
                        eng = DRAIN_ROT[(gg // 2) % len(DRAIN_ROT)]
                        if eng == "act":
                            nc.scalar.activation(
                                uhat[:, gg * F:(gg + 2) * F], ps[:],
                                mybir.ActivationFunctionType.Copy, scale=mulJ,
                            )
                        elif eng == "pool":
                            nc.gpsimd.tensor_scalar_mul(
                                uhat[:, gg * F:(gg + 2) * F], ps[:], mulJ
                            )
                        else:
                            nc.vector.tensor_scalar_mul(
                                uhat[:, gg * F:(gg + 2) * F], ps[:], mulJ
                            )

                # free the phase-1 streaming pools; routing pools reuse the space
                phase1.close()
                p_pool = ent(tc.tile_pool(name="ptree" + _sfx, bufs=P_BUFS))
                s_psum = ent(tc.tile_pool(name="sps" + _sfx, bufs=2, space="PSUM"))
                vr_psum = ent(tc.tile_pool(name="vrps" + _sfx, bufs=1, space="PSUM"))

                byp = mybir.AluOpType.bypass
                mul = mybir.AluOpType.mult
                add = mybir.AluOpType.add

                def squash(s_ps, out_dt, out_pool):
                    """s_ps: PSUM [BS, F] f32 in (e,j) layout -> V tile [BS, F]."""
                    # (hardware allows only one PSUM input per DVE op, so the
                    # square stays on ACT)
                    sqv = sq_pool.tile([BS, F], f32, tag="sqv")
                    nc.scalar.activation(
                        sqv[:], s_ps[:], mybir.ActivationFunctionType.Square
                    )
                    s2 = sq_pool.tile([BS, J], f32, tag="s2")
                    # reduce over e (outer dim): view (j, e) with e innermost
                    sq3 = sqv[:].rearrange("p (e j) -> p j e", e=E)
                    nc.vector.tensor_reduce(
                        s2[:], sq3, axis=mybir.AxisListType.X, op=add
                    )
                    # rt = sqrt(s2 + 1e-7) computed as exp(0.5*ln(s2 + 1e-7)).
                    # Ln and Exp share one ACT function set
                    # (natural_log_exp_and_others) so this avoids the
                    # ~1.3us LoadActFuncSet table reload Sqrt would incur
                    # at every routing-pass boundary.
                    lnv = sq_pool.tile([BS, J], f32, tag="lnv")
                    nc.scalar.activation(
                        lnv[:], s2[:], mybir.ActivationFunctionType.Ln, bias=1e-7
                    )
                    rt = sq_pool.tile([BS, J], f32, tag="rt")
                    nc.scalar.activation(
                        rt[:], lnv[:], mybir.ActivationFunctionType.Exp, scale=0.5
                    )
                    den = sq_pool.tile([BS, J], f32, tag="den")
                    nc.vector.scalar_tensor_tensor(
                        den[:], s2[:], 1.0, rt[:], op0=add, op1=mul
                    )
                    rden = sq_pool.tile([BS, J], f32, tag="rden")
                    nc.vector.reciprocal(rden[:], den[:])
                    sc = sq_pool.tile([BS, J], f32, tag="sc")
                    nc.vector.tensor_tensor(sc[:], s2[:], rden[:], op=mul)
                    # V = S * sc (broadcast sc over e)
                    vt = out_pool.tile([BS, F], out_dt, tag="vtile")
                    scb = sc[:][:, None, :].broadcast_to([BS, E, J])
                    nc.vector.scalar_tensor_tensor(
                        vt[:].rearrange("p (e j) -> p e j", e=E),
                        s_ps[:].rearrange("p (e j) -> p e j", e=E),
                        0.0, scb, op0=byp, op1=mul,
                    )
                    return vt

                def make_vrep(v_sb):
                    """v_sb [BS, F] fp16 -> V replicated to 128 partitions fp16."""
                    vr_ps = vr_psum.tile([128, F], f32)
                    nc.tensor.matmul(
                        vr_ps[:], lhsT=vind_sb[:], rhs=v_sb[:], start=True, stop=True
                    )
                    vr = vrep_pool.tile([128, F], fp16)
                    nc.scalar.activation(
                        vr[:], vr_ps[:], mybir.ActivationFunctionType.Copy
                    )
                    return vr

                def squash_rep(s_ps):
                    """Fused squash+replicate: the raw S is replicated to 128
                    partitions by PE while the squash scale is computed from
                    the PSUM S on the side; one final TT applies the scale.
                    Shortens the serial pass-boundary chain by ~1.3us."""
                    # raw S -> fp16 SBUF, then replicate via indicator matmul
                    s_sb = sq_pool.tile([BS, F], fp16, tag="ssb")
                    nc.scalar.activation(
                        s_sb[:], s_ps[:], mybir.ActivationFunctionType.Copy
                    )
                    sr_ps = vr_psum.tile([128, F], f32, tag="srp")
                    nc.tensor.matmul(
                        sr_ps[:], lhsT=vind_sb[:], rhs=s_sb[:], start=True,
                        stop=True,
                    )
                    # squash scale sc[b,j] on the 8-partition S (concurrent
                    # with the replicate matmul)
                    sqv = sq_pool.tile([BS, F], f32, tag="sqv")
                    nc.scalar.activation(
                        sqv[:], s_ps[:], mybir.ActivationFunctionType.Square
                    )
                    s2 = sq_pool.tile([BS, J], f32, tag="s2")
                    sq3 = sqv[:].rearrange("p (e j) -> p j e", e=E)
                    nc.vector.tensor_reduce(
                        s2[:], sq3, axis=mybir.AxisListType.X, op=add
                    )
                    lnv = sq_pool.tile([BS, J], f32, tag="lnv")
                    nc.scalar.activation(
                        lnv[:], s2[:], mybir.ActivationFunctionType.Ln, bias=1e-7
                    )
                    rt = sq_pool.tile([BS, J], f32, tag="rt")
                    nc.scalar.activation(
                        rt[:], lnv[:], mybir.ActivationFunctionType.Exp, scale=0.5
                    )
                    den = sq_pool.tile([BS, J], f32, tag="den")
                    nc.vector.scalar_tensor_tensor(
                        den[:], s2[:], 1.0, rt[:], op0=add, op1=mul
                    )
                    rden = sq_pool.tile([BS, J], f32, tag="rden")
                    nc.vector.reciprocal(rden[:], den[:])
                    sc16 = sq_pool.tile([BS, J], fp16, tag="sc16")
                    nc.vector.tensor_tensor(sc16[:], s2[:], rden[:], op=mul)
                    # replicate the scale and apply it to the replicated S.
                    # scp drains to SBUF first: the final TT may read only
                    # one of its inputs (sr_ps) from PSUM.
                    scp = vr_psum.tile([128, J], f32, tag="scp")
                    nc.tensor.matmul(
                        scp[:], lhsT=vind_sb[:], rhs=sc16[:], start=True,
                        stop=True,
                    )
                    sc128 = sq_pool.tile([128, J], f32, tag="sc128")
                    nc.scalar.activation(
                        sc128[:], scp[:], mybir.ActivationFunctionType.Copy
                    )
                    vr = vrep_pool.tile([128, F], fp16)
                    scb = sc128[:][:, None, :].broadcast_to([128, E, J])
                    nc.vector.tensor_tensor(
                        vr[:].rearrange("p (e j) -> p e j", e=E),
                        sr_ps[:].rearrange("p (e j) -> p e j", e=E),
                        scb, op=mul,
                    )
                    return vr

                n_mac = g_ // GB
                exp_f = mybir.ActivationFunctionType.Exp
                ln_f = mybir.ActivationFunctionType.Ln

                def routing_pass(vr, it, s_ps):
                    """One routing iteration: logits update, softmax, S matmul.

                    All large DVE ops are TensorTensor (2x mode for packed fp16).
                    The e-reduction tree runs in place inside the product tile.
                    Two-stage software pipeline: stage A (P, tree, a, exp) of
                    macro m+1 is emitted before stage B (sumexp, c, T, S-matmuls)
                    of macro m so DVE never stalls on the ACT exp.
                    """
                    def _macro_engines(m):
                        """(tt_engine, pg, dg) for macro m: whole-macro Pool
                        assignment via GP_MACRO, else g-split via GP_GSPLIT."""
                        if GP_MACRO and GP_MACRO[m % len(GP_MACRO)]:
                            return nc.gpsimd, 0, GB
                        return nc.vector, (
                            GP_GSPLIT[m % len(GP_GSPLIT)] if GP_GSPLIT else 0
                        ), None

                    def stage_a_tt(m):
                        g0 = m * GB
                        eng, pg, _ = _macro_engines(m)
                        dg = GB - pg
                        u8 = uv[:, g0:g0 + GB]                       # [128, GB, F]
                        u8e = u8.rearrange("p g (e j) -> p g e j", e=E)
                        # P = u_hat * V_rep  (TT, 2x on DVE; either trailing pg
                        # groups or the whole macro can run on GPSIMD instead)
                        p8 = p_pool.tile([128, GB * F], fp16)
                        p8v = p8[:].rearrange("p (g e j) -> p g e j", g=GB, e=E)
                        vrb = vr[:][:, None, :].broadcast_to([128, GB, F]).rearrange(
                            "p g (e j) -> p g e j", e=E
                        )
                        def _tt(outv, in0, in1, op):
                            # pool ops first so the (slower) Pool engine gets
                            # its work queued ahead of DVE's
                            if pg:
                                nc.gpsimd.tensor_tensor(
                                    outv[:, dg:], in0[:, dg:], in1[:, dg:], op=op
                                )
                            if dg:
                                eng.tensor_tensor(
                                    outv[:, :dg], in0[:, :dg], in1[:, :dg], op=op
                                )
                        _tt(p8v, u8e, vrb, mul)
                        # e-reduction tree 16->8->4->2->1, in place in p8
                        _tt(p8v[:, :, 0:8], p8v[:, :, 0:8], p8v[:, :, 8:16], add)
                        _tt(p8v[:, :, 0:4], p8v[:, :, 0:4], p8v[:, :, 4:8], add)
                        _tt(p8v[:, :, 0:2], p8v[:, :, 0:2], p8v[:, :, 2:4], add)
                        # logits — split per owning engine so DVE never waits
                        # on Pool's tree output (and vice versa)
                        def _tt2(outv, in0, in1, op):
                            if not SPLIT_SMALLS or not pg:
                                eng.tensor_tensor(outv, in0, in1, op=op)
                                return
                            if dg:
                                eng.tensor_tensor(
                                    outv[:, :dg], in0[:, :dg], in1[:, :dg], op=op
                                )
                            nc.gpsimd.tensor_tensor(
                                outv[:, dg:], in0[:, dg:], in1[:, dg:], op=op
                            )
                        if it == 1:
                            lg4v = a1v[:, g0:g0 + GB]                # write a1 in place
                            _tt2(lg4v, p8v[:, :, 0], p8v[:, :, 1], add)
                            if nonzero_b0:
                                _tt2(lg4v, lg4v, b0v[:, g0:g0 + GB], add)
                        else:
                            a2 = sm_pool.tile([128, GB * J], fp16, tag="a2")
                            a2v = a2[:].rearrange("p (g j) -> p g j", g=GB)
                            _tt2(a2v, p8v[:, :, 0], p8v[:, :, 1], add)
                            lg = sm_pool.tile([128, GB * J], fp16, tag="lg")
                            lg4v = lg[:].rearrange("p (g j) -> p g j", g=GB)
                            _tt2(lg4v, a2v, a1v[:, g0:g0 + GB], add)
                        return p8, u8e, lg4v

                    def stage_exp(m, lg4v):
                        # softmax over j, without max-subtraction: logits are
                        # bounded (|b| < ~25 for this distribution), so f32 exp
                        # is safe, and per-group ACT exps accumulate sumexp.
                        ex = sm_pool.tile([128, GB * J], f32, tag="ex")
                        exv = ex[:].rearrange("p (g j) -> p g j", g=GB)
                        se = sm_pool.tile([128, GB], f32, tag="se")
                        for k in range(GB):
                            nc.scalar.activation(
                                exv[:, k], lg4v[:, k], exp_f,
                                accum_out=se[:][:, k:k + 1],
                            )
                        return ex, se

                    def stage_b(m, p8, u8e, lgv, ex, se, first, last):
                        eng, pg, _ = _macro_engines(m)
                        if GP_STAGEA_ONLY and eng is nc.gpsimd:
                            eng, pg = nc.vector, 0
                        dg = GB - pg
                        p8v = p8[:].rearrange("p (g e j) -> p g e j", g=GB, e=E)
                        exv = ex[:].rearrange("p (g j) -> p g j", g=GB)
                        # rc/cc/T split per owning engine; the Pool-group rc
                        # stays on DVE (no Pool reciprocal) but is emitted
                        # after the big DVE T-mult so DVE doesn't stall on
                        # Pool's exps.
                        rc = sm_pool.tile([128, GB], f32, tag="rc")
                        cc = sm_pool.tile([128, GB * J], fp16, tag="cc")
                        ccv = cc[:].rearrange("p (g j) -> p g j", g=GB)
                        rcb = rc[:][:, :, None].broadcast_to([128, GB, J])
                        ccb = cc[:].rearrange("p (g j) -> p g j", g=GB)[
                            :, :, None, :
                        ].broadcast_to([128, GB, E, J])
                        if SPLIT_SMALLS and pg:
                            if dg:
                                nc.vector.reciprocal(
                                    rc[:][:, 0:dg], se[:][:, 0:dg]
                                )
                                eng.tensor_tensor(
                                    ccv[:, :dg], exv[:, :dg], rcb[:, :dg], op=mul
                                )
                                eng.tensor_tensor(
                                    p8v[:, :dg], u8e[:, :dg], ccb[:, :dg], op=mul
                                )
                            nc.vector.reciprocal(rc[:][:, dg:], se[:][:, dg:])
                            nc.gpsimd.tensor_tensor(
                                ccv[:, dg:], exv[:, dg:], rcb[:, dg:], op=mul
                            )
                            nc.gpsimd.tensor_tensor(
                                p8v[:, dg:], u8e[:, dg:], ccb[:, dg:], op=mul
                            )
                        elif CC_VIA_ACT:
                            # c = exp(lg - ln(sumexp)): trades the DVE
                            # reciprocal+multiply for ACT exps (ACT has slack;
                            # DVE is the routing bottleneck).  c <= 1 so the
                            # fp16 output cannot overflow.
                            lnse = sm_pool.tile([128, GB], f32, tag="lnse")
                            nc.scalar.activation(lnse[:], se[:], ln_f)
                            nls = sm_pool.tile([128, GB], f32, tag="nls")
                            nc.vector.tensor_scalar_mul(nls[:], lnse[:], -1.0)
                            for k in range(GB):
                                nc.scalar.activation(
                                    ccv[:, k], lgv[:, k], exp_f,
                                    bias=nls[:][:, k:k + 1],
                                )
                            if dg:
                                eng.tensor_tensor(
                                    p8v[:, :dg], u8e[:, :dg], ccb[:, :dg], op=mul
                                )
                            if pg:
                                nc.gpsimd.tensor_tensor(
                                    p8v[:, dg:], u8e[:, dg:], ccb[:, dg:], op=mul
                                )
                        else:
                            nc.vector.reciprocal(rc[:], se[:])
                            eng.tensor_tensor(ccv, exv, rcb, op=mul)
                            if dg:
                                eng.tensor_tensor(
                                    p8v[:, :dg], u8e[:, :dg], ccb[:, :dg], op=mul
                                )
                            if pg:
                                nc.gpsimd.tensor_tensor(
                                    p8v[:, dg:], u8e[:, dg:], ccb[:, dg:], op=mul
                                )
                        # PSUM accumulation is order-independent, so start/stop
                        # follow emission order, not g-order
                        for k in range(GB):
                            nc.tensor.matmul(
                                s_ps[:], lhsT=ind_sb[:],
                                rhs=p8[:, k * F:(k + 1) * F],
                                start=(first and k == 0),
                                stop=(last and k == GB - 1),
                            )

                    # Emission order: Pool-assigned macros evenly spaced among
                    # DVE macros; 2-deep slot pipeline (A at slot j, exps at
                    # j+1, B at j+2) so the slow Pool chains and ACT exps are
                    # never head-of-line blockers for the DVE stream.
                    pool_ms = [
                        m for m in range(n_mac)
                        if GP_MACRO and GP_MACRO[m % len(GP_MACRO)]
                    ]
                    dve_ms = [m for m in range(n_mac) if m not in pool_ms]
                    order = []
                    if pool_ms:
                        ratio = len(dve_ms) / len(pool_ms)
                        pi = 0
                        for i, m in enumerate(dve_ms):
                            order.append(m)
                            while pi < len(pool_ms) and (i + 1) >= (pi + 1) * ratio:
                                order.append(pool_ms[pi])
                                pi += 1
                        order.extend(pool_ms[pi:])
                    else:
                        order = dve_ms
                    n_ord = len(order)
                    states = {}
                    sb = 0
                    dep = PIPE_DEPTH
                    es = EXP_SLOT
                    for j in range(n_ord + dep):
                        if j < n_ord:
                            m = order[j]
                            states[m] = list(stage_a_tt(m))
                        if es <= j < n_ord + es:
                            m = order[j - es]
                            states[m] += list(stage_exp(m, states[m][2]))
                        if j >= dep:
                            m = order[j - dep]
                            p8, u8e, lgv, ex, se = states.pop(m)
                            stage_b(m, p8, u8e, lgv, ex, se,
                                    first=(sb == 0), last=(sb == n_ord - 1))
                            sb += 1

                # ---- routing ----
                if n_passes == 0:
                    vfin = squash(s0, f32, sq_pool)
                elif n_passes == 1:
                    v0 = squash(s0, fp16, sq_pool)
                    vr0 = make_vrep(v0)
                    s1 = s_psum.tile([BS, F], f32, tag="spsum")
                    routing_pass(vr0, 1, s1)
                    vfin = squash(s1, f32, sq_pool)
                else:
                    vr0 = squash_rep(s0)
                    s1 = s_psum.tile([BS, F], f32, tag="spsum")
                    routing_pass(vr0, 1, s1)
                    vr1 = squash_rep(s1)
                    s2_ps = s_psum.tile([BS, F], f32, tag="spsum")
                    routing_pass(vr1, 2, s2_ps)
                    vfin = squash(s2_ps, f32, sq_pool)
                nc.sync.dma_start(v_out, vfin[:])

    nc.compile()
    return nc


def _prep_inputs(inputs, W, b0, n_groups):
    """Host-side data layout. Returns (in_maps, nonzero_b0)."""
    g_ = n_groups
    i_ = g_ * IL
    nonzero_b0 = bool(np.any(b0[:i_]))

    w = np.ascontiguousarray(W[:i_]).astype(np.float32)
    # [i, j, d, e] -> [g, il, d, e, j] -> [g, 128, 512]
    wp = (
        w.reshape(g_, IL, J, D, E)
        .transpose(0, 1, 3, 4, 2)
        .reshape(g_, 128, J * E)
        .astype(np.float16)
    )

    shared = {"wp": wp}
    if nonzero_b0:
        c0 = b0[:i_].astype(np.float64)
        c0 = np.exp(c0 - c0.max(axis=1, keepdims=True))
        c0 = (c0 / c0.sum(axis=1, keepdims=True)).astype(np.float32)  # [i, J]
        # the on-chip S0 matmul uses xs0 = x/J as lhsT, so scale by J here
        w0 = w.reshape(g_, IL, J, D, E) * (c0 * J).reshape(g_, IL, J, 1, 1)
        wp0 = (
            w0.transpose(0, 1, 3, 4, 2).reshape(g_, 128, J * E).astype(np.float16)
        )
        shared["wp0"] = wp0
        # row-wise max-shift keeps the on-chip exp (no max subtraction) safe
        b0s = b0[:i_] - b0[:i_].max(axis=1, keepdims=True)
        b0p = np.broadcast_to(
            b0s.reshape(g_, IL, 1, J), (g_, IL, BS, J)
        )  # [g, il, b, j] ; partition = il*8+b
        shared["b0p"] = (
            np.ascontiguousarray(b0p.transpose(1, 2, 0, 3))
            .reshape(128, g_ * J)
            .astype(np.float32)
        )

    eye = np.eye(BS, dtype=np.float16)
    shared["ind"] = np.tile(eye, (IL, 1))          # [128, 8]
    shared["vind"] = shared["ind"].T.copy()        # [8, 128]

    shared["msk"] = np.kron(
        np.eye(IL, dtype=np.float16), np.ones((D, BS), np.float16)
    )  # [128, 128], 1 where il == il2

    in_maps = []
    for c in range(NC_CORES):
        xc = inputs[c * BS:(c + 1) * BS, :i_].astype(np.float32)  # [8, i, d]
        xt = xc.reshape(BS, g_, IL, D).transpose(1, 2, 3, 0)      # [g, il, d, b]
        xs0 = (
            np.ascontiguousarray(xt.transpose(1, 2, 0, 3)).reshape(128, g_ * BS)
            / np.float32(J)
        ).astype(np.float16).reshape(128, g_, BS)
        in_maps.append(dict(shared, xs0=xs0))
    return in_maps, nonzero_b0


def _get_program(n_groups, nonzero_b0):
    key = (n_groups, nonzero_b0)
    if key not in _CACHE:
        _CACHE[key] = _build_program(n_groups, nonzero_b0)
    return _CACHE[key]


def run_on_hw(inputs, W, b0, n_groups=G, trace=False):
    from concourse.bass_utils import run_bass_kernel_spmd

    in_maps, nonzero_b0 = _prep_inputs(inputs, W, b0, n_groups)
    nc = _get_program(n_groups, nonzero_b0)
    res = run_bass_kernel_spmd(nc, in_maps, list(range(NC_CORES)), trace=trace)
    outs = []
    for c in range(NC_CORES):
        v = res.results[c]["v_out"]                # [BS, 512] f32, (e,j) layout
        outs.append(v.reshape(BS, E, J).transpose(0, 2, 1))  # [BS, J, E]
    return np.concatenate(outs, axis=0).astype(np.float32), res


def kernel(inputs, W, b0):
    inputs = np.asarray(inputs, dtype=np.float32)
    W = np.asarray(W, dtype=np.float32)
    b0 = np.asarray(b0, dtype=np.float32)
    out, _ = run_on_hw(inputs, W, b0)
    return out



# revision 55
# speedup vs baseline: 1.0066x; 1.0066x over previous
"""CapsuleLayer dynamic-routing kernel for 8 Trainium2 NeuronCores.

Problem: u_hat[b,i,j,e] = einsum('bid,ijde->bije', x, W) with
B=64, I=2304, D=8, J=32, E=16, followed by NUM_ROUTING=3 softmax
routing iterations.  Output V = squash(S_2) with shape [B, J, E].

Strategy (data-parallel over batch, 8 b per core):
 - Host pre-lays W into fp16 tiles [G=144, 128, 512] with partition
   p = (i_loc*8 + d) and free f = (e*32 + j); W streams through SBUF once.
 - A block-diagonal lhsT (built on-chip from x/J with a mask multiply)
   makes ONE matmul produce u_hat for 16 i's x 8 b x (all j,e) per group;
   a second matmul per group accumulates S_0 = (1/J) sum_i u_hat in PSUM.
 - u_hat stays resident in SBUF as fp16 [128=(il,b), G*512=(g,(e,j))] —
   it never round-trips to HBM.
 - Each routing iteration (DVE TensorTensor ops run in 2x mode for packed
   fp16): P = u_hat*V_rep, in-place e-reduction tree for the agreement
   logits, one batched ACT exp for softmax, T = u_hat*c into the dead P
   tile, and PE contracts over i with a constant indicator lhsT,
   accumulating S in PSUM.  Two-stage software pipelining plus triple
   buffering of the product tile keeps DVE (the bottleneck) saturated.
"""

import sys

import numpy as np

sys.path.insert(0, "/opt/trn_rl_repo")

B, I, D, J, E = 64, 2304, 8, 32, 16
NC_CORES = 8
BS = B // NC_CORES          # 8 batch elements per core
IL = 16                     # i's per group
G = I // IL                 # 144 groups
F = J * E                   # 512 free elements per group
GB = 6                      # groups per batched DVE macro-op
P_BUFS = 5                  # product-tile buffering
W_BUFS = 4                  # W-stream buffering
SM_BUFS = 2                 # softmax small-tile buffering
GP_EVERY = 0                # offload P-mult of every Nth macro to GPSIMD (0=off)
# Pool offload: per-macro number of trailing groups whose P/tree/T ops run
# on GPSIMD instead of DVE (list cycled over macros; 0 = all-DVE).
GP_GSPLIT = (1, 1, 1, 2) * 5 + (2, 1, 1, 1)
# Whole-macro Pool assignment: cycled truth-list; macros marked 1 run their
# entire TT chain (P, tree, logit-add, cc, T) on GPSIMD so the slower Pool
# engine is never on the DVE pipeline's critical path.  (Measured worse than
# the fine-grained GP_GSPLIT on this problem; left available but off.)
GP_MACRO = ()
GP_STAGEA_ONLY = True       # Pool macros offload only the P/tree/logit chain;
                            # cc/T (feeding PE S-matmuls) stay on DVE
SPLIT_SMALLS = False        # split a2/lg/rc/cc per owning engine
CC_VIA_ACT = False          # c = exp(lg - ln(sumexp)) on ACT instead of
                            # reciprocal+multiply on (bottleneck) DVE
                            # (measured worse: ACT becomes the bottleneck)
PIPE_DEPTH = 1              # stage_b lag (slots) behind stage_a
EXP_SLOT = 0                # exp emission lag (slots) behind stage_a
# phase-1 psum-drain engine rotation (GPSIMD cannot access PSUM, so only
# ACT/DVE are legal here)
DRAIN_ROT = ("act", "dve", "act")
LT_ENG = "alt"              # engine for phase-1 block-diagonal lhsT builds
L_BUFS = 8                  # lhsT tile ring depth
NUM_ROUTING = 3

_CACHE = {}


def _force_single_act_table(bacc, mybir):
    """Constrain the activation-table chooser to one set covering every
    function this kernel uses (Exp/Ln/Copy/Square), so no LoadActFuncSet
    reloads (~1.3us each on ACT) appear at routing-pass boundaries.  Set ids
    stay positional, so all other sets are emptied rather than removed."""
    if getattr(bacc, "_act_tables_forced", False):
        return
    orig = bacc.get_activation_tables
    AF = mybir.ActivationFunctionType
    need = {AF.Exp, AF.Ln, AF.Copy, AF.Square}

    def patched(arch):
        tabs = orig(arch)
        chosen = None
        for name, fns in tabs.items():
            if need <= set(fns):
                chosen = name
                break
        if chosen is not None:
            for name in list(tabs):
                if name != chosen:
                    tabs[name] = set()
        return tabs

    bacc.get_activation_tables = patched
    bacc._act_tables_forced = True


def _build_program(n_groups, nonzero_b0, n_passes=2, n_bodies=1):
    import concourse.bass as bass
    import concourse.mybir as mybir
    import concourse.tile as tile
    from concourse import bacc

    _force_single_act_table(bacc, mybir)

    fp16 = mybir.dt.float16
    bf16 = mybir.dt.bfloat16
    f32 = mybir.dt.float32

    nc = bacc.Bacc("TRN2", target_bir_lowering=False, debug=False)

    # register the squash-epsilon constant for activation bias
    eps_t = nc.alloc_sbuf_tensor("const-f32-eps", [128, 1], f32)
    nc.gpsimd.memset(eps_t.ap(), 1e-7)
    nc.const_aps.aps[(f32, 1e-7)] = eps_t.ap()
    nc.all_engine_barrier()

    g_ = n_groups
    wp = nc.dram_tensor("wp", [g_, 128, F], fp16, kind="ExternalInput").ap()
    xs0 = nc.dram_tensor("xs0", [128, g_, BS], fp16, kind="ExternalInput").ap()
    msk = nc.dram_tensor("msk", [128, 128], fp16, kind="ExternalInput").ap()
    ind = nc.dram_tensor("ind", [128, BS], fp16, kind="ExternalInput").ap()
    vind = nc.dram_tensor("vind", [BS, 128], fp16, kind="ExternalInput").ap()
    if nonzero_b0:
        wp0 = nc.dram_tensor("wp0", [g_, 128, F], fp16, kind="ExternalInput").ap()
        b0p = nc.dram_tensor("b0p", [128, g_ * J], f32, kind="ExternalInput").ap()
    v_out = nc.dram_tensor("v_out", [BS, F], f32, kind="ExternalOutput").ap()

    from contextlib import ExitStack

    with tile.TileContext(nc) as tc:
        for _body in range(n_bodies):
            _sfx = "" if _body == 0 else "@%d" % _body
            with ExitStack() as ctx:
                ent = ctx.enter_context
                uhat_pool = ent(tc.tile_pool(name="uhat" + _sfx, bufs=1))
                alog_pool = ent(tc.tile_pool(name="alog" + _sfx, bufs=1))
                cst_pool = ent(tc.tile_pool(name="cst" + _sfx, bufs=1))
                sm_pool = ent(tc.tile_pool(name="sm" + _sfx, bufs=SM_BUFS))
                vrep_pool = ent(tc.tile_pool(name="vrep" + _sfx, bufs=2))
                sq_pool = ent(tc.tile_pool(name="sq" + _sfx, bufs=1))
                s0_psum = ent(tc.tile_pool(name="s0ps" + _sfx, bufs=1, space="PSUM"))
                phase1 = ExitStack()
                xs0_pool = phase1.enter_context(tc.tile_pool(name="xs0p" + _sfx, bufs=1))
                w_pool = phase1.enter_context(tc.tile_pool(name="wstream" + _sfx, bufs=W_BUFS))
                l_pool = phase1.enter_context(
                    tc.tile_pool(name="lstream" + _sfx, bufs=L_BUFS)
                )
                mm_psum = phase1.enter_context(
                    tc.tile_pool(name="mmps" + _sfx, bufs=3, space="PSUM")
                )
                # ---- persistent SBUF tensors ----
                uhat = uhat_pool.tile([128, g_ * F], fp16)       # (g,(e,j)) per part
                uv = uhat[:].rearrange("p (g f) -> p g f", g=g_)
                # iteration-1 logits in fp16: |logit| < ~30 so the absolute
                # quantization step is <= 0.016; the induced ~1.6% relative
                # c-weight noise averages out over I=2304 in S.  Keeping the
                # tile fp16 makes the a2/lg adds 2x-mode and frees 9KB of
                # SBUF per partition for deeper product-tile buffering.
                a1 = alog_pool.tile([128, g_ * J], fp16)
                a1v = a1[:].rearrange("p (g j) -> p g j", g=g_)
                xs0_sb = xs0_pool.tile([128, g_ * BS], fp16)
                xs0v = xs0_sb[:].rearrange("p (g b) -> p g b", g=g_)
                ind_sb = cst_pool.tile([128, BS], fp16)
                vind_sb = cst_pool.tile([BS, 128], fp16)
                msk_sb = cst_pool.tile([128, 128], fp16)
                if nonzero_b0:
                    b0_sb = alog_pool.tile([128, g_ * J], f32)
                    b0v = b0_sb[:].rearrange("p (g j) -> p g j", g=g_)

                nc.sync.dma_start(xs0_sb[:], xs0.rearrange("p g b -> p (g b)"))
                nc.sync.dma_start(ind_sb[:], ind)
                nc.sync.dma_start(vind_sb[:], vind)
                nc.sync.dma_start(msk_sb[:], msk)
                if nonzero_b0:
                    nc.sync.dma_start(b0_sb[:], b0p)

                # ---- phase 1: u_hat + S0 ----
                # W DMA in batches of GD groups.  The block-diagonal lhsT is
                # built on-chip from xs0 (= x/J) with a mask multiply, so u_hat
                # lands in PSUM scaled by 1/J and the psum->SBUF copy multiplies
                # by J.  Copies alternate between ACT and DVE.
                GD = 8
                mulJ = float(J)
                s0 = s0_psum.tile([BS, F], f32)
                assert g_ % GD == 0
                _mm = mybir.AluOpType.mult
                for gd in range(g_ // GD):
                    g0 = gd * GD
                    wt = w_pool.tile([128, GD * F], fp16)
                    wtv = wt[:].rearrange("p (g f) -> p g f", g=GD)
                    nc.sync.dma_start(wtv, wp[g0:g0 + GD].rearrange("g p f -> p g f"))
                    if nonzero_b0:
                        w0t = w_pool.tile([128, GD * F], fp16, tag="w0t")
                        w0tv = w0t[:].rearrange("p (g f) -> p g f", g=GD)
                        nc.sync.dma_start(
                            w0tv, wp0[g0:g0 + GD].rearrange("g p f -> p g f")
                        )
                    for h in range(GD // 2):
                        ps = mm_psum.tile([128, 2 * F], f32)
                        for k in range(2):
                            g = g0 + h * 2 + k
                            lt = l_pool.tile([128, 128], fp16)
                            xsb = xs0v[:, g][:, None, :].broadcast_to([128, IL, BS])
                            _lt_eng = (
                                nc.gpsimd if LT_ENG == "pool"
                                else nc.vector if LT_ENG == "dve"
                                else (nc.gpsimd if g % 2 else nc.vector)
                            )
                            _lt_eng.tensor_tensor(
                                lt[:].rearrange("p (i b) -> p i b", i=IL),
                                xsb, msk_sb[:].rearrange("p (i b) -> p i b", i=IL),
                                op=_mm,
                            )
                            nc.tensor.matmul(
                                ps[:, k * F:(k + 1) * F], lhsT=lt[:],
                                rhs=wtv[:, h * 2 + k], start=True, stop=True,
                            )
                            s0_rhs = w0tv[:, h * 2 + k] if nonzero_b0 else wtv[:, h * 2 + k]
                            nc.tensor.matmul(
                                s0[:], lhsT=xs0v[:, g], rhs=s0_rhs,
                                start=(g == 0), stop=(g == g_ - 1),
                            )
                        gg = g0 + h * 2
                        eng = DRAIN_ROT[(gg // 2) % len(DRAIN_ROT)]
                        if eng == "act":
                            nc.scalar.activation(
                                uhat[:, gg * F:(gg + 2) * F], ps[:],
                                mybir.ActivationFunctionType.Copy, scale=mulJ,
                            )
                        elif eng == "pool":
                            nc.gpsimd.tensor_scalar_mul(
                                uhat[:, gg * F:(gg + 2) * F], ps[:], mulJ
                            )
                        else:
                            nc.vector.tensor_scalar_mul(
                                uhat[:, gg * F:(gg + 2) * F], ps[:], mulJ
                            )

                # free the phase-1 streaming pools; routing pools reuse the space
                phase1.close()
                p_pool = ent(tc.tile_pool(name="ptree" + _sfx, bufs=P_BUFS))
                s_psum = ent(tc.tile_pool(name="sps" + _sfx, bufs=2, space="PSUM"))
                vr_psum = ent(tc.tile_pool(name="vrps" + _sfx, bufs=1, space="PSUM"))

                byp = mybir.AluOpType.bypass
                mul = mybir.AluOpType.mult
                add = mybir.AluOpType.add

                def squash(s_ps, out_dt, out_pool):
                    """s_ps: PSUM [BS, F] f32 in (e,j) layout -> V tile [BS, F]."""
                    # (hardware allows only one PSUM input per DVE op, so the
                    # square stays on ACT)
                    sqv = sq_pool.tile([BS, F], f32, tag="sqv")
                    nc.scalar.activation(
                        sqv[:], s_ps[:], mybir.ActivationFunctionType.Square
                    )
                    s2 = sq_pool.tile([BS, J], f32, tag="s2")
                    # reduce over e (outer dim): view (j, e) with e innermost
                    sq3 = sqv[:].rearrange("p (e j) -> p j e", e=E)
                    nc.vector.tensor_reduce(
                        s2[:], sq3, axis=mybir.AxisListType.X, op=add
                    )
                    # rt = sqrt(s2 + 1e-7) computed as exp(0.5*ln(s2 + 1e-7)).
                    # Ln and Exp share one ACT function set
                    # (natural_log_exp_and_others) so this avoids the
                    # ~1.3us LoadActFuncSet table reload Sqrt would incur
                    # at every routing-pass boundary.
                    lnv = sq_pool.tile([BS, J], f32, tag="lnv")
                    nc.scalar.activation(
                        lnv[:], s2[:], mybir.ActivationFunctionType.Ln, bias=1e-7
                    )
                    rt = sq_pool.tile([BS, J], f32, tag="rt")
                    nc.scalar.activation(
                        rt[:], lnv[:], mybir.ActivationFunctionType.Exp, scale=0.5
                    )
                    den = sq_pool.tile([BS, J], f32, tag="den")
                    nc.vector.scalar_tensor_tensor(
                        den[:], s2[:], 1.0, rt[:], op0=add, op1=mul
                    )
                    rden = sq_pool.tile([BS, J], f32, tag="rden")
                    nc.vector.reciprocal(rden[:], den[:])
                    sc = sq_pool.tile([BS, J], f32, tag="sc")
                    nc.vector.tensor_tensor(sc[:], s2[:], rden[:], op=mul)
                    # V = S * sc (broadcast sc over e)
                    vt = out_pool.tile([BS, F], out_dt, tag="vtile")
                    scb = sc[:][:, None, :].broadcast_to([BS, E, J])
                    nc.vector.scalar_tensor_tensor(
                        vt[:].rearrange("p (e j) -> p e j", e=E),
                        s_ps[:].rearrange("p (e j) -> p e j", e=E),
                        0.0, scb, op0=byp, op1=mul,
                    )
                    return vt

                def make_vrep(v_sb):
                    """v_sb [BS, F] fp16 -> V replicated to 128 partitions fp16."""
                    vr_ps = vr_psum.tile([128, F], f32)
                    nc.tensor.matmul(
                        vr_ps[:], lhsT=vind_sb[:], rhs=v_sb[:], start=True, stop=True
                    )
                    vr = vrep_pool.tile([128, F], fp16)
                    nc.scalar.activation(
                        vr[:], vr_ps[:], mybir.ActivationFunctionType.Copy
                    )
                    return vr

                def squash_rep(s_ps):
                    """Fused squash+replicate: the raw S is replicated to 128
                    partitions by PE while the squash scale is computed from
                    the PSUM S on the side; one final TT applies the scale.
                    Shortens the serial pass-boundary chain by ~1.3us."""
                    # The Square gates the long scale chain, so it is emitted
                    # first on ACT; the raw-S copy feeding the (short)
                    # replicate branch follows it.
                    sqv = sq_pool.tile([BS, F], f32, tag="sqv")
                    nc.scalar.activation(
                        sqv[:], s_ps[:], mybir.ActivationFunctionType.Square
                    )
                    s_sb = sq_pool.tile([BS, F], fp16, tag="ssb")
                    nc.scalar.activation(
                        s_sb[:], s_ps[:], mybir.ActivationFunctionType.Copy
                    )
                    sr_ps = vr_psum.tile([128, F], f32, tag="srp")
                    nc.tensor.matmul(
                        sr_ps[:], lhsT=vind_sb[:], rhs=s_sb[:], start=True,
                        stop=True,
                    )
                    s2 = sq_pool.tile([BS, J], f32, tag="s2")
                    sq3 = sqv[:].rearrange("p (e j) -> p j e", e=E)
                    nc.vector.tensor_reduce(
                        s2[:], sq3, axis=mybir.AxisListType.X, op=add
                    )
                    lnv = sq_pool.tile([BS, J], f32, tag="lnv")
                    nc.scalar.activation(
                        lnv[:], s2[:], mybir.ActivationFunctionType.Ln, bias=1e-7
                    )
                    rt = sq_pool.tile([BS, J], f32, tag="rt")
                    nc.scalar.activation(
                        rt[:], lnv[:], mybir.ActivationFunctionType.Exp, scale=0.5
                    )
                    den = sq_pool.tile([BS, J], f32, tag="den")
                    nc.vector.scalar_tensor_tensor(
                        den[:], s2[:], 1.0, rt[:], op0=add, op1=mul
                    )
                    rden = sq_pool.tile([BS, J], f32, tag="rden")
                    nc.vector.reciprocal(rden[:], den[:])
                    sc16 = sq_pool.tile([BS, J], fp16, tag="sc16")
                    nc.vector.tensor_tensor(sc16[:], s2[:], rden[:], op=mul)
                    # replicate the scale and apply it to the replicated S.
                    # scp drains to SBUF first: the final TT may read only
                    # one of its inputs (sr_ps) from PSUM.
                    scp = vr_psum.tile([128, J], f32, tag="scp")
                    nc.tensor.matmul(
                        scp[:], lhsT=vind_sb[:], rhs=sc16[:], start=True,
                        stop=True,
                    )
                    sc128 = sq_pool.tile([128, J], f32, tag="sc128")
                    nc.scalar.activation(
                        sc128[:], scp[:], mybir.ActivationFunctionType.Copy
                    )
                    vr = vrep_pool.tile([128, F], fp16)
                    scb = sc128[:][:, None, :].broadcast_to([128, E, J])
                    nc.vector.tensor_tensor(
                        vr[:].rearrange("p (e j) -> p e j", e=E),
                        sr_ps[:].rearrange("p (e j) -> p e j", e=E),
                        scb, op=mul,
                    )
                    return vr

                n_mac = g_ // GB
                exp_f = mybir.ActivationFunctionType.Exp
                ln_f = mybir.ActivationFunctionType.Ln

                def routing_pass(vr, it, s_ps):
                    """One routing iteration: logits update, softmax, S matmul.

                    All large DVE ops are TensorTensor (2x mode for packed fp16).
                    The e-reduction tree runs in place inside the product tile.
                    Two-stage software pipeline: stage A (P, tree, a, exp) of
                    macro m+1 is emitted before stage B (sumexp, c, T, S-matmuls)
                    of macro m so DVE never stalls on the ACT exp.
                    """
                    def _macro_engines(m):
                        """(tt_engine, pg, dg) for macro m: whole-macro Pool
                        assignment via GP_MACRO, else g-split via GP_GSPLIT."""
                        if GP_MACRO and GP_MACRO[m % len(GP_MACRO)]:
                            return nc.gpsimd, 0, GB
                        return nc.vector, (
                            GP_GSPLIT[m % len(GP_GSPLIT)] if GP_GSPLIT else 0
                        ), None

                    def stage_a_tt(m):
                        g0 = m * GB
                        eng, pg, _ = _macro_engines(m)
                        dg = GB - pg
                        u8 = uv[:, g0:g0 + GB]                       # [128, GB, F]
                        u8e = u8.rearrange("p g (e j) -> p g e j", e=E)
                        # P = u_hat * V_rep  (TT, 2x on DVE; either trailing pg
                        # groups or the whole macro can run on GPSIMD instead)
                        p8 = p_pool.tile([128, GB * F], fp16)
                        p8v = p8[:].rearrange("p (g e j) -> p g e j", g=GB, e=E)
                        vrb = vr[:][:, None, :].broadcast_to([128, GB, F]).rearrange(
                            "p g (e j) -> p g e j", e=E
                        )
                        def _tt(outv, in0, in1, op):
                            # pool ops first so the (slower) Pool engine gets
                            # its work queued ahead of DVE's
                            if pg:
                                nc.gpsimd.tensor_tensor(
                                    outv[:, dg:], in0[:, dg:], in1[:, dg:], op=op
                                )
                            if dg:
                                eng.tensor_tensor(
                                    outv[:, :dg], in0[:, :dg], in1[:, :dg], op=op
                                )
                        _tt(p8v, u8e, vrb, mul)
                        # e-reduction tree 16->8->4->2->1, in place in p8
                        _tt(p8v[:, :, 0:8], p8v[:, :, 0:8], p8v[:, :, 8:16], add)
                        _tt(p8v[:, :, 0:4], p8v[:, :, 0:4], p8v[:, :, 4:8], add)
                        _tt(p8v[:, :, 0:2], p8v[:, :, 0:2], p8v[:, :, 2:4], add)
                        # logits — split per owning engine so DVE never waits
                        # on Pool's tree output (and vice versa)
                        def _tt2(outv, in0, in1, op):
                            if not SPLIT_SMALLS or not pg:
                                eng.tensor_tensor(outv, in0, in1, op=op)
                                return
                            if dg:
                                eng.tensor_tensor(
                                    outv[:, :dg], in0[:, :dg], in1[:, :dg], op=op
                                )
                            nc.gpsimd.tensor_tensor(
                                outv[:, dg:], in0[:, dg:], in1[:, dg:], op=op
                            )
                        if it == 1:
                            lg4v = a1v[:, g0:g0 + GB]                # write a1 in place
                            _tt2(lg4v, p8v[:, :, 0], p8v[:, :, 1], add)
                            if nonzero_b0:
                                _tt2(lg4v, lg4v, b0v[:, g0:g0 + GB], add)
                        else:
                            a2 = sm_pool.tile([128, GB * J], fp16, tag="a2")
                            a2v = a2[:].rearrange("p (g j) -> p g j", g=GB)
                            _tt2(a2v, p8v[:, :, 0], p8v[:, :, 1], add)
                            lg = sm_pool.tile([128, GB * J], fp16, tag="lg")
                            lg4v = lg[:].rearrange("p (g j) -> p g j", g=GB)
                            _tt2(lg4v, a2v, a1v[:, g0:g0 + GB], add)
                        return p8, u8e, lg4v

                    def stage_exp(m, lg4v):
                        # softmax over j, without max-subtraction: logits are
                        # bounded (|b| < ~25 for this distribution), so f32 exp
                        # is safe, and per-group ACT exps accumulate sumexp.
                        ex = sm_pool.tile([128, GB * J], f32, tag="ex")
                        exv = ex[:].rearrange("p (g j) -> p g j", g=GB)
                        se = sm_pool.tile([128, GB], f32, tag="se")
                        for k in range(GB):
                            nc.scalar.activation(
                                exv[:, k], lg4v[:, k], exp_f,
                                accum_out=se[:][:, k:k + 1],
                            )
                        return ex, se

                    def stage_b(m, p8, u8e, lgv, ex, se, first, last):
                        eng, pg, _ = _macro_engines(m)
                        if GP_STAGEA_ONLY and eng is nc.gpsimd:
                            eng, pg = nc.vector, 0
                        dg = GB - pg
                        p8v = p8[:].rearrange("p (g e j) -> p g e j", g=GB, e=E)
                        exv = ex[:].rearrange("p (g j) -> p g j", g=GB)
                        # rc/cc/T split per owning engine; the Pool-group rc
                        # stays on DVE (no Pool reciprocal) but is emitted
                        # after the big DVE T-mult so DVE doesn't stall on
                        # Pool's exps.
                        rc = sm_pool.tile([128, GB], f32, tag="rc")
                        cc = sm_pool.tile([128, GB * J], fp16, tag="cc")
                        ccv = cc[:].rearrange("p (g j) -> p g j", g=GB)
                        rcb = rc[:][:, :, None].broadcast_to([128, GB, J])
                        ccb = cc[:].rearrange("p (g j) -> p g j", g=GB)[
                            :, :, None, :
                        ].broadcast_to([128, GB, E, J])
                        if SPLIT_SMALLS and pg:
                            if dg:
                                nc.vector.reciprocal(
                                    rc[:][:, 0:dg], se[:][:, 0:dg]
                                )
                                eng.tensor_tensor(
                                    ccv[:, :dg], exv[:, :dg], rcb[:, :dg], op=mul
                                )
                                eng.tensor_tensor(
                                    p8v[:, :dg], u8e[:, :dg], ccb[:, :dg], op=mul
                                )
                            nc.vector.reciprocal(rc[:][:, dg:], se[:][:, dg:])
                            nc.gpsimd.tensor_tensor(
                                ccv[:, dg:], exv[:, dg:], rcb[:, dg:], op=mul
                            )
                            nc.gpsimd.tensor_tensor(
                                p8v[:, dg:], u8e[:, dg:], ccb[:, dg:], op=mul
                            )
                        elif CC_VIA_ACT:
                            # c = exp(lg - ln(sumexp)): trades the DVE
                            # reciprocal+multiply for ACT exps (ACT has slack;
                            # DVE is the routing bottleneck).  c <= 1 so the
                            # fp16 output cannot overflow.
                            lnse = sm_pool.tile([128, GB], f32, tag="lnse")
                            nc.scalar.activation(lnse[:], se[:], ln_f)
                            nls = sm_pool.tile([128, GB], f32, tag="nls")
                            nc.vector.tensor_scalar_mul(nls[:], lnse[:], -1.0)
                            for k in range(GB):
                                nc.scalar.activation(
                                    ccv[:, k], lgv[:, k], exp_f,
                                    bias=nls[:][:, k:k + 1],
                                )
                            if dg:
                                eng.tensor_tensor(
                                    p8v[:, :dg], u8e[:, :dg], ccb[:, :dg], op=mul
                                )
                            if pg:
                                nc.gpsimd.tensor_tensor(
                                    p8v[:, dg:], u8e[:, dg:], ccb[:, dg:], op=mul
                                )
                        else:
                            nc.vector.reciprocal(rc[:], se[:])
                            eng.tensor_tensor(ccv, exv, rcb, op=mul)
                            if dg:
                                eng.tensor_tensor(
                                    p8v[:, :dg], u8e[:, :dg], ccb[:, :dg], op=mul
                                )
                            if pg:
                                nc.gpsimd.tensor_tensor(
                                    p8v[:, dg:], u8e[:, dg:], ccb[:, dg:], op=mul
                                )
                        # PSUM accumulation is order-independent, so start/stop
                        # follow emission order, not g-order
                        for k in range(GB):
                            nc.tensor.matmul(
                                s_ps[:], lhsT=ind_sb[:],
                                rhs=p8[:, k * F:(k + 1) * F],
                                start=(first and k == 0),
                                stop=(last and k == GB - 1),
                            )

                    # Emission order: Pool-assigned macros evenly spaced among
                    # DVE macros; 2-deep slot pipeline (A at slot j, exps at
                    # j+1, B at j+2) so the slow Pool chains and ACT exps are
                    # never head-of-line blockers for the DVE stream.
                    pool_ms = [
                        m for m in range(n_mac)
                        if GP_MACRO and GP_MACRO[m % len(GP_MACRO)]
                    ]
                    dve_ms = [m for m in range(n_mac) if m not in pool_ms]
                    order = []
                    if pool_ms:
                        ratio = len(dve_ms) / len(pool_ms)
                        pi = 0
                        for i, m in enumerate(dve_ms):
                            order.append(m)
                            while pi < len(pool_ms) and (i + 1) >= (pi + 1) * ratio:
                                order.append(pool_ms[pi])
                                pi += 1
                        order.extend(pool_ms[pi:])
                    else:
                        order = dve_ms
                    n_ord = len(order)
                    states = {}
                    sb = 0
                    dep = PIPE_DEPTH
                    es = EXP_SLOT
                    for j in range(n_ord + dep):
                        if j < n_ord:
                            m = order[j]
                            states[m] = list(stage_a_tt(m))
                        if es <= j < n_ord + es:
                            m = order[j - es]
                            states[m] += list(stage_exp(m, states[m][2]))
                        if j >= dep:
                            m = order[j - dep]
                            p8, u8e, lgv, ex, se = states.pop(m)
                            stage_b(m, p8, u8e, lgv, ex, se,
                                    first=(sb == 0), last=(sb == n_ord - 1))
                            sb += 1

                # ---- routing ----
                if n_passes == 0:
                    vfin = squash(s0, f32, sq_pool)
                elif n_passes == 1:
                    v0 = squash(s0, fp16, sq_pool)
                    vr0 = make_vrep(v0)
                    s1 = s_psum.tile([BS, F], f32, tag="spsum")
                    routing_pass(vr0, 1, s1)
                    vfin = squash(s1, f32, sq_pool)
                else:
                    vr0 = squash_rep(s0)
                    s1 = s_psum.tile([BS, F], f32, tag="spsum")
                    routing_pass(vr0, 1, s1)
                    vr1 = squash_rep(s1)
                    s2_ps = s_psum.tile([BS, F], f32, tag="spsum")
                    routing_pass(vr1, 2, s2_ps)
                    vfin = squash(s2_ps, f32, sq_pool)
                nc.sync.dma_start(v_out, vfin[:])

    nc.compile()
    return nc


def _prep_inputs(inputs, W, b0, n_groups):
    """Host-side data layout. Returns (in_maps, nonzero_b0)."""
    g_ = n_groups
    i_ = g_ * IL
    nonzero_b0 = bool(np.any(b0[:i_]))

    w = np.ascontiguousarray(W[:i_]).astype(np.float32)
    # [i, j, d, e] -> [g, il, d, e, j] -> [g, 128, 512]
    wp = (
        w.reshape(g_, IL, J, D, E)
        .transpose(0, 1, 3, 4, 2)
        .reshape(g_, 128, J * E)
        .astype(np.float16)
    )

    shared = {"wp": wp}
    if nonzero_b0:
        c0 = b0[:i_].astype(np.float64)
        c0 = np.exp(c0 - c0.max(axis=1, keepdims=True))
        c0 = (c0 / c0.sum(axis=1, keepdims=True)).astype(np.float32)  # [i, J]
        # the on-chip S0 matmul uses xs0 = x/J as lhsT, so scale by J here
        w0 = w.reshape(g_, IL, J, D, E) * (c0 * J).reshape(g_, IL, J, 1, 1)
        wp0 = (
            w0.transpose(0, 1, 3, 4, 2).reshape(g_, 128, J * E).astype(np.float16)
        )
        shared["wp0"] = wp0
        # row-wise max-shift keeps the on-chip exp (no max subtraction) safe
        b0s = b0[:i_] - b0[:i_].max(axis=1, keepdims=True)
        b0p = np.broadcast_to(
            b0s.reshape(g_, IL, 1, J), (g_, IL, BS, J)
        )  # [g, il, b, j] ; partition = il*8+b
        shared["b0p"] = (
            np.ascontiguousarray(b0p.transpose(1, 2, 0, 3))
            .reshape(128, g_ * J)
            .astype(np.float32)
        )

    eye = np.eye(BS, dtype=np.float16)
    shared["ind"] = np.tile(eye, (IL, 1))          # [128, 8]
    shared["vind"] = shared["ind"].T.copy()        # [8, 128]

    shared["msk"] = np.kron(
        np.eye(IL, dtype=np.float16), np.ones((D, BS), np.float16)
    )  # [128, 128], 1 where il == il2

    in_maps = []
    for c in range(NC_CORES):
        xc = inputs[c * BS:(c + 1) * BS, :i_].astype(np.float32)  # [8, i, d]
        xt = xc.reshape(BS, g_, IL, D).transpose(1, 2, 3, 0)      # [g, il, d, b]
        xs0 = (
            np.ascontiguousarray(xt.transpose(1, 2, 0, 3)).reshape(128, g_ * BS)
            / np.float32(J)
        ).astype(np.float16).reshape(128, g_, BS)
        in_maps.append(dict(shared, xs0=xs0))
    return in_maps, nonzero_b0


def _get_program(n_groups, nonzero_b0):
    key = (n_groups, nonzero_b0)
    if key not in _CACHE:
        _CACHE[key] = _build_program(n_groups, nonzero_b0)
    return _CACHE[key]


def run_on_hw(inputs, W, b0, n_groups=G, trace=False):
    from concourse.bass_utils import run_bass_kernel_spmd

    in_maps, nonzero_b0 = _prep_inputs(inputs, W, b0, n_groups)
    nc = _get_program(n_groups, nonzero_b0)
    res = run_bass_kernel_spmd(nc, in_maps, list(range(NC_CORES)), trace=trace)
    outs = []
    for c in range(NC_CORES):
        v = res.results[c]["v_out"]                # [BS, 512] f32, (e,j) layout
        outs.append(v.reshape(BS, E, J).transpose(0, 2, 1))  # [BS, J, E]
    return np.concatenate(outs, axis=0).astype(np.float32), res


def kernel(inputs, W, b0):
    inputs = np.asarray(inputs, dtype=np.float32)
    W = np.asarray(W, dtype=np.float32)
    b0 = np.asarray(b0, dtype=np.float32)
    out, _ = run_on_hw(inputs, W, b0)
    return out



# revision 58
# speedup vs baseline: 1.0090x; 1.0024x over previous
"""CapsuleLayer dynamic-routing kernel for 8 Trainium2 NeuronCores.

Problem: u_hat[b,i,j,e] = einsum('bid,ijde->bije', x, W) with
B=64, I=2304, D=8, J=32, E=16, followed by NUM_ROUTING=3 softmax
routing iterations.  Output V = squash(S_2) with shape [B, J, E].

Strategy (data-parallel over batch, 8 b per core):
 - Host pre-lays W into fp16 tiles [G=144, 128, 512] with partition
   p = (i_loc*8 + d) and free f = (e*32 + j); W streams through SBUF once.
 - A block-diagonal lhsT (built on-chip from x/J with a mask multiply)
   makes ONE matmul produce u_hat for 16 i's x 8 b x (all j,e) per group;
   a second matmul per group accumulates S_0 = (1/J) sum_i u_hat in PSUM.
 - u_hat stays resident in SBUF as fp16 [128=(il,b), G*512=(g,(e,j))] —
   it never round-trips to HBM.
 - Each routing iteration (DVE TensorTensor ops run in 2x mode for packed
   fp16): P = u_hat*V_rep, in-place e-reduction tree for the agreement
   logits, one batched ACT exp for softmax, T = u_hat*c into the dead P
   tile, and PE contracts over i with a constant indicator lhsT,
   accumulating S in PSUM.  Two-stage software pipelining plus triple
   buffering of the product tile keeps DVE (the bottleneck) saturated.
"""

import sys

import numpy as np

sys.path.insert(0, "/opt/trn_rl_repo")

B, I, D, J, E = 64, 2304, 8, 32, 16
NC_CORES = 8
BS = B // NC_CORES          # 8 batch elements per core
IL = 16                     # i's per group
G = I // IL                 # 144 groups
F = J * E                   # 512 free elements per group
GB = 6                      # groups per batched DVE macro-op
P_BUFS = 5                  # product-tile buffering
W_BUFS = 4                  # W-stream buffering
SM_BUFS = 2                 # softmax small-tile buffering
GP_EVERY = 0                # offload P-mult of every Nth macro to GPSIMD (0=off)
# Pool offload: per-macro number of trailing groups whose P/tree/T ops run
# on GPSIMD instead of DVE (list cycled over macros; 0 = all-DVE).
GP_GSPLIT = (1, 1, 1, 2) * 5 + (2, 1, 1, 1)
# Whole-macro Pool assignment: cycled truth-list; macros marked 1 run their
# entire TT chain (P, tree, logit-add, cc, T) on GPSIMD so the slower Pool
# engine is never on the DVE pipeline's critical path.  (Measured worse than
# the fine-grained GP_GSPLIT on this problem; left available but off.)
GP_MACRO = ()
GP_STAGEA_ONLY = True       # Pool macros offload only the P/tree/logit chain;
                            # cc/T (feeding PE S-matmuls) stay on DVE
SPLIT_SMALLS = False        # split a2/lg/rc/cc per owning engine
CC_VIA_ACT = False          # c = exp(lg - ln(sumexp)) on ACT instead of
                            # reciprocal+multiply on (bottleneck) DVE
                            # (measured worse: ACT becomes the bottleneck)
PIPE_DEPTH = 1              # stage_b lag (slots) behind stage_a
EXP_SLOT = 0                # exp emission lag (slots) behind stage_a
# phase-1 psum-drain engine rotation (GPSIMD cannot access PSUM, so only
# ACT/DVE are legal here)
DRAIN_ROT = ("act", "act", "dve")
LT_ENG = "alt"              # engine for phase-1 block-diagonal lhsT builds
L_BUFS = 8                  # lhsT tile ring depth
GD_BATCH = 8                # groups per W-stream DMA batch
NUM_ROUTING = 3

_CACHE = {}


def _force_single_act_table(bacc, mybir):
    """Constrain the activation-table chooser to one set covering every
    function this kernel uses (Exp/Ln/Copy/Square), so no LoadActFuncSet
    reloads (~1.3us each on ACT) appear at routing-pass boundaries.  Set ids
    stay positional, so all other sets are emptied rather than removed."""
    if getattr(bacc, "_act_tables_forced", False):
        return
    orig = bacc.get_activation_tables
    AF = mybir.ActivationFunctionType
    need = {AF.Exp, AF.Ln, AF.Copy, AF.Square}

    def patched(arch):
        tabs = orig(arch)
        chosen = None
        for name, fns in tabs.items():
            if need <= set(fns):
                chosen = name
                break
        if chosen is not None:
            for name in list(tabs):
                if name != chosen:
                    tabs[name] = set()
        return tabs

    bacc.get_activation_tables = patched
    bacc._act_tables_forced = True


def _build_program(n_groups, nonzero_b0, n_passes=2, n_bodies=1):
    import concourse.bass as bass
    import concourse.mybir as mybir
    import concourse.tile as tile
    from concourse import bacc

    _force_single_act_table(bacc, mybir)

    fp16 = mybir.dt.float16
    bf16 = mybir.dt.bfloat16
    f32 = mybir.dt.float32

    nc = bacc.Bacc("TRN2", target_bir_lowering=False, debug=False)

    # register the squash-epsilon constant for activation bias
    eps_t = nc.alloc_sbuf_tensor("const-f32-eps", [128, 1], f32)
    nc.gpsimd.memset(eps_t.ap(), 1e-7)
    nc.const_aps.aps[(f32, 1e-7)] = eps_t.ap()
    nc.all_engine_barrier()

    g_ = n_groups
    wp = nc.dram_tensor("wp", [g_, 128, F], fp16, kind="ExternalInput").ap()
    xs0 = nc.dram_tensor("xs0", [128, g_, BS], fp16, kind="ExternalInput").ap()
    msk = nc.dram_tensor("msk", [128, 128], fp16, kind="ExternalInput").ap()
    ind = nc.dram_tensor("ind", [128, BS], fp16, kind="ExternalInput").ap()
    vind = nc.dram_tensor("vind", [BS, 128], fp16, kind="ExternalInput").ap()
    if nonzero_b0:
        wp0 = nc.dram_tensor("wp0", [g_, 128, F], fp16, kind="ExternalInput").ap()
        b0p = nc.dram_tensor("b0p", [128, g_ * J], f32, kind="ExternalInput").ap()
    v_out = nc.dram_tensor("v_out", [BS, F], f32, kind="ExternalOutput").ap()

    from contextlib import ExitStack

    with tile.TileContext(nc) as tc:
        for _body in range(n_bodies):
            _sfx = "" if _body == 0 else "@%d" % _body
            with ExitStack() as ctx:
                ent = ctx.enter_context
                uhat_pool = ent(tc.tile_pool(name="uhat" + _sfx, bufs=1))
                alog_pool = ent(tc.tile_pool(name="alog" + _sfx, bufs=1))
                cst_pool = ent(tc.tile_pool(name="cst" + _sfx, bufs=1))
                sm_pool = ent(tc.tile_pool(name="sm" + _sfx, bufs=SM_BUFS))
                vrep_pool = ent(tc.tile_pool(name="vrep" + _sfx, bufs=2))
                sq_pool = ent(tc.tile_pool(name="sq" + _sfx, bufs=1))
                s0_psum = ent(tc.tile_pool(name="s0ps" + _sfx, bufs=1, space="PSUM"))
                phase1 = ExitStack()
                xs0_pool = phase1.enter_context(tc.tile_pool(name="xs0p" + _sfx, bufs=1))
                w_pool = phase1.enter_context(tc.tile_pool(name="wstream" + _sfx, bufs=W_BUFS))
                l_pool = phase1.enter_context(
                    tc.tile_pool(name="lstream" + _sfx, bufs=L_BUFS)
                )
                mm_psum = phase1.enter_context(
                    tc.tile_pool(name="mmps" + _sfx, bufs=3, space="PSUM")
                )
                # ---- persistent SBUF tensors ----
                uhat = uhat_pool.tile([128, g_ * F], fp16)       # (g,(e,j)) per part
                uv = uhat[:].rearrange("p (g f) -> p g f", g=g_)
                # iteration-1 logits in fp16: |logit| < ~30 so the absolute
                # quantization step is <= 0.016; the induced ~1.6% relative
                # c-weight noise averages out over I=2304 in S.  Keeping the
                # tile fp16 makes the a2/lg adds 2x-mode and frees 9KB of
                # SBUF per partition for deeper product-tile buffering.
                a1 = alog_pool.tile([128, g_ * J], fp16)
                a1v = a1[:].rearrange("p (g j) -> p g j", g=g_)
                xs0_sb = xs0_pool.tile([128, g_ * BS], fp16)
                xs0v = xs0_sb[:].rearrange("p (g b) -> p g b", g=g_)
                ind_sb = cst_pool.tile([128, BS], fp16)
                vind_sb = cst_pool.tile([BS, 128], fp16)
                msk_sb = cst_pool.tile([128, 128], fp16)
                if nonzero_b0:
                    b0_sb = alog_pool.tile([128, g_ * J], f32)
                    b0v = b0_sb[:].rearrange("p (g j) -> p g j", g=g_)

                nc.sync.dma_start(xs0_sb[:], xs0.rearrange("p g b -> p (g b)"))
                nc.sync.dma_start(ind_sb[:], ind)
                nc.sync.dma_start(vind_sb[:], vind)
                nc.sync.dma_start(msk_sb[:], msk)
                if nonzero_b0:
                    nc.sync.dma_start(b0_sb[:], b0p)

                # ---- phase 1: u_hat + S0 ----
                # W DMA in batches of GD groups.  The block-diagonal lhsT is
                # built on-chip from xs0 (= x/J) with a mask multiply, so u_hat
                # lands in PSUM scaled by 1/J and the psum->SBUF copy multiplies
                # by J.  Copies alternate between ACT and DVE.
                GD = GD_BATCH
                mulJ = float(J)
                s0 = s0_psum.tile([BS, F], f32)
                assert g_ % GD == 0
                _mm = mybir.AluOpType.mult
                for gd in range(g_ // GD):
                    g0 = gd * GD
                    wt = w_pool.tile([128, GD * F], fp16)
                    wtv = wt[:].rearrange("p (g f) -> p g f", g=GD)
                    nc.sync.dma_start(wtv, wp[g0:g0 + GD].rearrange("g p f -> p g f"))
                    if nonzero_b0:
                        w0t = w_pool.tile([128, GD * F], fp16, tag="w0t")
                        w0tv = w0t[:].rearrange("p (g f) -> p g f", g=GD)
                        nc.sync.dma_start(
                            w0tv, wp0[g0:g0 + GD].rearrange("g p f -> p g f")
                        )
                    for h in range(GD // 2):
                        ps = mm_psum.tile([128, 2 * F], f32)
                        for k in range(2):
                            g = g0 + h * 2 + k
                            lt = l_pool.tile([128, 128], fp16)
                            xsb = xs0v[:, g][:, None, :].broadcast_to([128, IL, BS])
                            _lt_eng = (
                                nc.gpsimd if LT_ENG == "pool"
                                else nc.vector if LT_ENG == "dve"
                                else (nc.gpsimd if g % 2 else nc.vector)
                            )
                            _lt_eng.tensor_tensor(
                                lt[:].rearrange("p (i b) -> p i b", i=IL),
                                xsb, msk_sb[:].rearrange("p (i b) -> p i b", i=IL),
                                op=_mm,
                            )
                            nc.tensor.matmul(
                                ps[:, k * F:(k + 1) * F], lhsT=lt[:],
                                rhs=wtv[:, h * 2 + k], start=True, stop=True,
                            )
                            s0_rhs = w0tv[:, h * 2 + k] if nonzero_b0 else wtv[:, h * 2 + k]
                            nc.tensor.matmul(
                                s0[:], lhsT=xs0v[:, g], rhs=s0_rhs,
                                start=(g == 0), stop=(g == g_ - 1),
                            )
                        gg = g0 + h * 2
                        eng = DRAIN_ROT[(gg // 2) % len(DRAIN_ROT)]
                        if eng == "act":
                            nc.scalar.activation(
                                uhat[:, gg * F:(gg + 2) * F], ps[:],
                                mybir.ActivationFunctionType.Copy, scale=mulJ,
                            )
                        elif eng == "pool":
                            nc.gpsimd.tensor_scalar_mul(
                                uhat[:, gg * F:(gg + 2) * F], ps[:], mulJ
                            )
                        else:
                            nc.vector.tensor_scalar_mul(
                                uhat[:, gg * F:(gg + 2) * F], ps[:], mulJ
                            )

                # free the phase-1 streaming pools; routing pools reuse the space
                phase1.close()
                p_pool = ent(tc.tile_pool(name="ptree" + _sfx, bufs=P_BUFS))
                s_psum = ent(tc.tile_pool(name="sps" + _sfx, bufs=2, space="PSUM"))
                vr_psum = ent(tc.tile_pool(name="vrps" + _sfx, bufs=1, space="PSUM"))

                byp = mybir.AluOpType.bypass
                mul = mybir.AluOpType.mult
                add = mybir.AluOpType.add

                def squash(s_ps, out_dt, out_pool):
                    """s_ps: PSUM [BS, F] f32 in (e,j) layout -> V tile [BS, F]."""
                    # (hardware allows only one PSUM input per DVE op, so the
                    # square stays on ACT)
                    sqv = sq_pool.tile([BS, F], f32, tag="sqv")
                    nc.scalar.activation(
                        sqv[:], s_ps[:], mybir.ActivationFunctionType.Square
                    )
                    s2 = sq_pool.tile([BS, J], f32, tag="s2")
                    # reduce over e (outer dim): view (j, e) with e innermost
                    sq3 = sqv[:].rearrange("p (e j) -> p j e", e=E)
                    nc.vector.tensor_reduce(
                        s2[:], sq3, axis=mybir.AxisListType.X, op=add
                    )
                    # rt = sqrt(s2 + 1e-7) computed as exp(0.5*ln(s2 + 1e-7)).
                    # Ln and Exp share one ACT function set
                    # (natural_log_exp_and_others) so this avoids the
                    # ~1.3us LoadActFuncSet table reload Sqrt would incur
                    # at every routing-pass boundary.
                    lnv = sq_pool.tile([BS, J], f32, tag="lnv")
                    nc.scalar.activation(
                        lnv[:], s2[:], mybir.ActivationFunctionType.Ln, bias=1e-7
                    )
                    rt = sq_pool.tile([BS, J], f32, tag="rt")
                    nc.scalar.activation(
                        rt[:], lnv[:], mybir.ActivationFunctionType.Exp, scale=0.5
                    )
                    den = sq_pool.tile([BS, J], f32, tag="den")
                    nc.vector.scalar_tensor_tensor(
                        den[:], s2[:], 1.0, rt[:], op0=add, op1=mul
                    )
                    rden = sq_pool.tile([BS, J], f32, tag="rden")
                    nc.vector.reciprocal(rden[:], den[:])
                    sc = sq_pool.tile([BS, J], f32, tag="sc")
                    nc.vector.tensor_tensor(sc[:], s2[:], rden[:], op=mul)
                    # V = S * sc (broadcast sc over e)
                    vt = out_pool.tile([BS, F], out_dt, tag="vtile")
                    scb = sc[:][:, None, :].broadcast_to([BS, E, J])
                    nc.vector.scalar_tensor_tensor(
                        vt[:].rearrange("p (e j) -> p e j", e=E),
                        s_ps[:].rearrange("p (e j) -> p e j", e=E),
                        0.0, scb, op0=byp, op1=mul,
                    )
                    return vt

                def make_vrep(v_sb):
                    """v_sb [BS, F] fp16 -> V replicated to 128 partitions fp16."""
                    vr_ps = vr_psum.tile([128, F], f32)
                    nc.tensor.matmul(
                        vr_ps[:], lhsT=vind_sb[:], rhs=v_sb[:], start=True, stop=True
                    )
                    vr = vrep_pool.tile([128, F], fp16)
                    nc.scalar.activation(
                        vr[:], vr_ps[:], mybir.ActivationFunctionType.Copy
                    )
                    return vr

                def squash_rep(s_ps):
                    """Fused squash+replicate: the raw S is replicated to 128
                    partitions by PE while the squash scale is computed from
                    the PSUM S on the side; one final TT applies the scale.
                    Shortens the serial pass-boundary chain by ~1.3us."""
                    # The Square gates the long scale chain, so it is emitted
                    # first on ACT; the raw-S copy feeding the (short)
                    # replicate branch follows it.
                    sqv = sq_pool.tile([BS, F], f32, tag="sqv")
                    nc.scalar.activation(
                        sqv[:], s_ps[:], mybir.ActivationFunctionType.Square
                    )
                    s_sb = sq_pool.tile([BS, F], fp16, tag="ssb")
                    nc.scalar.activation(
                        s_sb[:], s_ps[:], mybir.ActivationFunctionType.Copy
                    )
                    sr_ps = vr_psum.tile([128, F], f32, tag="srp")
                    nc.tensor.matmul(
                        sr_ps[:], lhsT=vind_sb[:], rhs=s_sb[:], start=True,
                        stop=True,
                    )
                    s2 = sq_pool.tile([BS, J], f32, tag="s2")
                    sq3 = sqv[:].rearrange("p (e j) -> p j e", e=E)
                    nc.vector.tensor_reduce(
                        s2[:], sq3, axis=mybir.AxisListType.X, op=add
                    )
                    lnv = sq_pool.tile([BS, J], f32, tag="lnv")
                    nc.scalar.activation(
                        lnv[:], s2[:], mybir.ActivationFunctionType.Ln, bias=1e-7
                    )
                    rt = sq_pool.tile([BS, J], f32, tag="rt")
                    nc.scalar.activation(
                        rt[:], lnv[:], mybir.ActivationFunctionType.Exp, scale=0.5
                    )
                    den = sq_pool.tile([BS, J], f32, tag="den")
                    nc.vector.scalar_tensor_tensor(
                        den[:], s2[:], 1.0, rt[:], op0=add, op1=mul
                    )
                    rden = sq_pool.tile([BS, J], f32, tag="rden")
                    nc.vector.reciprocal(rden[:], den[:])
                    sc16 = sq_pool.tile([BS, J], fp16, tag="sc16")
                    nc.vector.tensor_tensor(sc16[:], s2[:], rden[:], op=mul)
                    # replicate the scale and apply it to the replicated S.
                    # scp drains to SBUF first: the final TT may read only
                    # one of its inputs (sr_ps) from PSUM.
                    scp = vr_psum.tile([128, J], f32, tag="scp")
                    nc.tensor.matmul(
                        scp[:], lhsT=vind_sb[:], rhs=sc16[:], start=True,
                        stop=True,
                    )
                    sc128 = sq_pool.tile([128, J], f32, tag="sc128")
                    nc.scalar.activation(
                        sc128[:], scp[:], mybir.ActivationFunctionType.Copy
                    )
                    vr = vrep_pool.tile([128, F], fp16)
                    scb = sc128[:][:, None, :].broadcast_to([128, E, J])
                    nc.vector.tensor_tensor(
                        vr[:].rearrange("p (e j) -> p e j", e=E),
                        sr_ps[:].rearrange("p (e j) -> p e j", e=E),
                        scb, op=mul,
                    )
                    return vr

                n_mac = g_ // GB
                exp_f = mybir.ActivationFunctionType.Exp
                ln_f = mybir.ActivationFunctionType.Ln

                def routing_pass(vr, it, s_ps):
                    """One routing iteration: logits update, softmax, S matmul.

                    All large DVE ops are TensorTensor (2x mode for packed fp16).
                    The e-reduction tree runs in place inside the product tile.
                    Two-stage software pipeline: stage A (P, tree, a, exp) of
                    macro m+1 is emitted before stage B (sumexp, c, T, S-matmuls)
                    of macro m so DVE never stalls on the ACT exp.
                    """
                    def _macro_engines(m):
                        """(tt_engine, pg, dg) for macro m: whole-macro Pool
                        assignment via GP_MACRO, else g-split via GP_GSPLIT."""
                        if GP_MACRO and GP_MACRO[m % len(GP_MACRO)]:
                            return nc.gpsimd, 0, GB
                        return nc.vector, (
                            GP_GSPLIT[m % len(GP_GSPLIT)] if GP_GSPLIT else 0
                        ), None

                    def stage_a_tt(m):
                        g0 = m * GB
                        eng, pg, _ = _macro_engines(m)
                        dg = GB - pg
                        u8 = uv[:, g0:g0 + GB]                       # [128, GB, F]
                        u8e = u8.rearrange("p g (e j) -> p g e j", e=E)
                        # P = u_hat * V_rep  (TT, 2x on DVE; either trailing pg
                        # groups or the whole macro can run on GPSIMD instead)
                        p8 = p_pool.tile([128, GB * F], fp16)
                        p8v = p8[:].rearrange("p (g e j) -> p g e j", g=GB, e=E)
                        vrb = vr[:][:, None, :].broadcast_to([128, GB, F]).rearrange(
                            "p g (e j) -> p g e j", e=E
                        )
                        def _tt(outv, in0, in1, op):
                            # pool ops first so the (slower) Pool engine gets
                            # its work queued ahead of DVE's
                            if pg:
                                nc.gpsimd.tensor_tensor(
                                    outv[:, dg:], in0[:, dg:], in1[:, dg:], op=op
                                )
                            if dg:
                                eng.tensor_tensor(
                                    outv[:, :dg], in0[:, :dg], in1[:, :dg], op=op
                                )
                        _tt(p8v, u8e, vrb, mul)
                        # e-reduction tree 16->8->4->2->1, in place in p8
                        _tt(p8v[:, :, 0:8], p8v[:, :, 0:8], p8v[:, :, 8:16], add)
                        _tt(p8v[:, :, 0:4], p8v[:, :, 0:4], p8v[:, :, 4:8], add)
                        _tt(p8v[:, :, 0:2], p8v[:, :, 0:2], p8v[:, :, 2:4], add)
                        # logits — split per owning engine so DVE never waits
                        # on Pool's tree output (and vice versa)
                        def _tt2(outv, in0, in1, op):
                            if not SPLIT_SMALLS or not pg:
                                eng.tensor_tensor(outv, in0, in1, op=op)
                                return
                            if dg:
                                eng.tensor_tensor(
                                    outv[:, :dg], in0[:, :dg], in1[:, :dg], op=op
                                )
                            nc.gpsimd.tensor_tensor(
                                outv[:, dg:], in0[:, dg:], in1[:, dg:], op=op
                            )
                        if it == 1:
                            lg4v = a1v[:, g0:g0 + GB]                # write a1 in place
                            _tt2(lg4v, p8v[:, :, 0], p8v[:, :, 1], add)
                            if nonzero_b0:
                                _tt2(lg4v, lg4v, b0v[:, g0:g0 + GB], add)
                        else:
                            a2 = sm_pool.tile([128, GB * J], fp16, tag="a2")
                            a2v = a2[:].rearrange("p (g j) -> p g j", g=GB)
                            _tt2(a2v, p8v[:, :, 0], p8v[:, :, 1], add)
                            lg = sm_pool.tile([128, GB * J], fp16, tag="lg")
                            lg4v = lg[:].rearrange("p (g j) -> p g j", g=GB)
                            _tt2(lg4v, a2v, a1v[:, g0:g0 + GB], add)
                        return p8, u8e, lg4v

                    def stage_exp(m, lg4v):
                        # softmax over j, without max-subtraction: logits are
                        # bounded (|b| < ~25 for this distribution), so f32 exp
                        # is safe, and per-group ACT exps accumulate sumexp.
                        ex = sm_pool.tile([128, GB * J], f32, tag="ex")
                        exv = ex[:].rearrange("p (g j) -> p g j", g=GB)
                        se = sm_pool.tile([128, GB], f32, tag="se")
                        for k in range(GB):
                            nc.scalar.activation(
                                exv[:, k], lg4v[:, k], exp_f,
                                accum_out=se[:][:, k:k + 1],
                            )
                        return ex, se

                    def stage_b(m, p8, u8e, lgv, ex, se, first, last):
                        eng, pg, _ = _macro_engines(m)
                        if GP_STAGEA_ONLY and eng is nc.gpsimd:
                            eng, pg = nc.vector, 0
                        dg = GB - pg
                        p8v = p8[:].rearrange("p (g e j) -> p g e j", g=GB, e=E)
                        exv = ex[:].rearrange("p (g j) -> p g j", g=GB)
                        # rc/cc/T split per owning engine; the Pool-group rc
                        # stays on DVE (no Pool reciprocal) but is emitted
                        # after the big DVE T-mult so DVE doesn't stall on
                        # Pool's exps.
                        rc = sm_pool.tile([128, GB], f32, tag="rc")
                        cc = sm_pool.tile([128, GB * J], fp16, tag="cc")
                        ccv = cc[:].rearrange("p (g j) -> p g j", g=GB)
                        rcb = rc[:][:, :, None].broadcast_to([128, GB, J])
                        ccb = cc[:].rearrange("p (g j) -> p g j", g=GB)[
                            :, :, None, :
                        ].broadcast_to([128, GB, E, J])
                        if SPLIT_SMALLS and pg:
                            if dg:
                                nc.vector.reciprocal(
                                    rc[:][:, 0:dg], se[:][:, 0:dg]
                                )
                                eng.tensor_tensor(
                                    ccv[:, :dg], exv[:, :dg], rcb[:, :dg], op=mul
                                )
                                eng.tensor_tensor(
                                    p8v[:, :dg], u8e[:, :dg], ccb[:, :dg], op=mul
                                )
                            nc.vector.reciprocal(rc[:][:, dg:], se[:][:, dg:])
                            nc.gpsimd.tensor_tensor(
                                ccv[:, dg:], exv[:, dg:], rcb[:, dg:], op=mul
                            )
                            nc.gpsimd.tensor_tensor(
                                p8v[:, dg:], u8e[:, dg:], ccb[:, dg:], op=mul
                            )
                        elif CC_VIA_ACT:
                            # c = exp(lg - ln(sumexp)): trades the DVE
                            # reciprocal+multiply for ACT exps (ACT has slack;
                            # DVE is the routing bottleneck).  c <= 1 so the
                            # fp16 output cannot overflow.
                            lnse = sm_pool.tile([128, GB], f32, tag="lnse")
                            nc.scalar.activation(lnse[:], se[:], ln_f)
                            nls = sm_pool.tile([128, GB], f32, tag="nls")
                            nc.vector.tensor_scalar_mul(nls[:], lnse[:], -1.0)
                            for k in range(GB):
                                nc.scalar.activation(
                                    ccv[:, k], lgv[:, k], exp_f,
                                    bias=nls[:][:, k:k + 1],
                                )
                            if dg:
                                eng.tensor_tensor(
                                    p8v[:, :dg], u8e[:, :dg], ccb[:, :dg], op=mul
                                )
                            if pg:
                                nc.gpsimd.tensor_tensor(
                                    p8v[:, dg:], u8e[:, dg:], ccb[:, dg:], op=mul
                                )
                        else:
                            nc.vector.reciprocal(rc[:], se[:])
                            eng.tensor_tensor(ccv, exv, rcb, op=mul)
                            if dg:
                                eng.tensor_tensor(
                                    p8v[:, :dg], u8e[:, :dg], ccb[:, :dg], op=mul
                                )
                            if pg:
                                nc.gpsimd.tensor_tensor(
                                    p8v[:, dg:], u8e[:, dg:], ccb[:, dg:], op=mul
                                )
                        # PSUM accumulation is order-independent, so start/stop
                        # follow emission order, not g-order
                        for k in range(GB):
                            nc.tensor.matmul(
                                s_ps[:], lhsT=ind_sb[:],
                                rhs=p8[:, k * F:(k + 1) * F],
                                start=(first and k == 0),
                                stop=(last and k == GB - 1),
                            )

                    # Emission order: Pool-assigned macros evenly spaced among
                    # DVE macros; 2-deep slot pipeline (A at slot j, exps at
                    # j+1, B at j+2) so the slow Pool chains and ACT exps are
                    # never head-of-line blockers for the DVE stream.
                    pool_ms = [
                        m for m in range(n_mac)
                        if GP_MACRO and GP_MACRO[m % len(GP_MACRO)]
                    ]
                    dve_ms = [m for m in range(n_mac) if m not in pool_ms]
                    order = []
                    if pool_ms:
                        ratio = len(dve_ms) / len(pool_ms)
                        pi = 0
                        for i, m in enumerate(dve_ms):
                            order.append(m)
                            while pi < len(pool_ms) and (i + 1) >= (pi + 1) * ratio:
                                order.append(pool_ms[pi])
                                pi += 1
                        order.extend(pool_ms[pi:])
                    else:
                        order = dve_ms
                    n_ord = len(order)
                    states = {}
                    sb = 0
                    dep = PIPE_DEPTH
                    es = EXP_SLOT
                    for j in range(n_ord + dep):
                        if j < n_ord:
                            m = order[j]
                            states[m] = list(stage_a_tt(m))
                        if es <= j < n_ord + es:
                            m = order[j - es]
                            states[m] += list(stage_exp(m, states[m][2]))
                        if j >= dep:
                            m = order[j - dep]
                            p8, u8e, lgv, ex, se = states.pop(m)
                            stage_b(m, p8, u8e, lgv, ex, se,
                                    first=(sb == 0), last=(sb == n_ord - 1))
                            sb += 1

                # ---- routing ----
                if n_passes == 0:
                    vfin = squash(s0, f32, sq_pool)
                elif n_passes == 1:
                    v0 = squash(s0, fp16, sq_pool)
                    vr0 = make_vrep(v0)
                    s1 = s_psum.tile([BS, F], f32, tag="spsum")
                    routing_pass(vr0, 1, s1)
                    vfin = squash(s1, f32, sq_pool)
                else:
                    vr0 = squash_rep(s0)
                    s1 = s_psum.tile([BS, F], f32, tag="spsum")
                    routing_pass(vr0, 1, s1)
                    vr1 = squash_rep(s1)
                    s2_ps = s_psum.tile([BS, F], f32, tag="spsum")
                    routing_pass(vr1, 2, s2_ps)
                    vfin = squash(s2_ps, f32, sq_pool)
                nc.sync.dma_start(v_out, vfin[:])

    nc.compile()
    return nc


def _prep_inputs(inputs, W, b0, n_groups):
    """Host-side data layout. Returns (in_maps, nonzero_b0)."""
    g_ = n_groups
    i_ = g_ * IL
    nonzero_b0 = bool(np.any(b0[:i_]))

    w = np.ascontiguousarray(W[:i_]).astype(np.float32)
    # [i, j, d, e] -> [g, il, d, e, j] -> [g, 128, 512]
    wp = (
        w.reshape(g_, IL, J, D, E)
        .transpose(0, 1, 3, 4, 2)
        .reshape(g_, 128, J * E)
        .astype(np.float16)
    )

    shared = {"wp": wp}
    if nonzero_b0:
        c0 = b0[:i_].astype(np.float64)
        c0 = np.exp(c0 - c0.max(axis=1, keepdims=True))
        c0 = (c0 / c0.sum(axis=1, keepdims=True)).astype(np.float32)  # [i, J]
        # the on-chip S0 matmul uses xs0 = x/J as lhsT, so scale by J here
        w0 = w.reshape(g_, IL, J, D, E) * (c0 * J).reshape(g_, IL, J, 1, 1)
        wp0 = (
            w0.transpose(0, 1, 3, 4, 2).reshape(g_, 128, J * E).astype(np.float16)
        )
        shared["wp0"] = wp0
        # row-wise max-shift keeps the on-chip exp (no max subtraction) safe
        b0s = b0[:i_] - b0[:i_].max(axis=1, keepdims=True)
        b0p = np.broadcast_to(
            b0s.reshape(g_, IL, 1, J), (g_, IL, BS, J)
        )  # [g, il, b, j] ; partition = il*8+b
        shared["b0p"] = (
            np.ascontiguousarray(b0p.transpose(1, 2, 0, 3))
            .reshape(128, g_ * J)
            .astype(np.float32)
        )

    eye = np.eye(BS, dtype=np.float16)
    shared["ind"] = np.tile(eye, (IL, 1))          # [128, 8]
    shared["vind"] = shared["ind"].T.copy()        # [8, 128]

    shared["msk"] = np.kron(
        np.eye(IL, dtype=np.float16), np.ones((D, BS), np.float16)
    )  # [128, 128], 1 where il == il2

    in_maps = []
    for c in range(NC_CORES):
        xc = inputs[c * BS:(c + 1) * BS, :i_].astype(np.float32)  # [8, i, d]
        xt = xc.reshape(BS, g_, IL, D).transpose(1, 2, 3, 0)      # [g, il, d, b]
        xs0 = (
            np.ascontiguousarray(xt.transpose(1, 2, 0, 3)).reshape(128, g_ * BS)
            / np.float32(J)
        ).astype(np.float16).reshape(128, g_, BS)
        in_maps.append(dict(shared, xs0=xs0))
    return in_maps, nonzero_b0


def _get_program(n_groups, nonzero_b0):
    key = (n_groups, nonzero_b0)
    if key not in _CACHE:
        _CACHE[key] = _build_program(n_groups, nonzero_b0)
    return _CACHE[key]


def run_on_hw(inputs, W, b0, n_groups=G, trace=False):
    from concourse.bass_utils import run_bass_kernel_spmd

    in_maps, nonzero_b0 = _prep_inputs(inputs, W, b0, n_groups)
    nc = _get_program(n_groups, nonzero_b0)
    res = run_bass_kernel_spmd(nc, in_maps, list(range(NC_CORES)), trace=trace)
    outs = []
    for c in range(NC_CORES):
        v = res.results[c]["v_out"]                # [BS, 512] f32, (e,j) layout
        outs.append(v.reshape(BS, E, J).transpose(0, 2, 1))  # [BS, J, E]
    return np.concatenate(outs, axis=0).astype(np.float32), res


def kernel(inputs, W, b0):
    inputs = np.asarray(inputs, dtype=np.float32)
    W = np.asarray(W, dtype=np.float32)
    b0 = np.asarray(b0, dtype=np.float32)
    out, _ = run_on_hw(inputs, W, b0)
    return out

